# revision 16
# baseline (speedup 1.0000x reference)
"""Causal single-head attention (B=4, T=4096, C=1024, H=64) on 8 TRN2 NeuronCores.

Sharding: 2 cores per batch element. Core s of a pair owns q blocks
OWN[s] = {0,3,4,7} / {1,2,5,6} (512 rows each), which balances the causal
workload (72 useful kv tiles per core) while keeping the SPMD instruction
stream identical across cores: slot index j handles global block OWN[s][j],
and the two parities' blocks for a given j always live in the same 1024-wide
x piece, so a single register offset ((j+s)%2)*512 selects the right Q slice
and causal-mask bank.

One SPMD program for all 8 cores:
  - x arrives pre-transposed per batch as [C, T]; loaded as [128, 1024]
    pieces, cast f32->f16 on the Pool engine (DVE/ACT are loaded).
  - K|V projection runs for the full batch on both cores of a pair; Q is
    projected only for the 4 owned blocks (register-selected x slice).
  - Attention computes S^T = K_tile^T @ Q per 128-wide kv tile: no max pass
    (scores bounded), no P transpose, row-sum folded into P@V via a ones
    column in V. exp runs on ACT over TWO kv tiles at once ([128, 2, 512]
    PSUM spanning 2 banks) with the 1/sqrt(H) scale fused; causal masking is
    a multiplicative f16 mask on the last 8 kv tiles of each q block, bank
    chosen by the (j+s)%2 register.
  - Emission is software-pipelined: the S matmuls for kv pair n+1 are issued
    before the exp/mask/PV drain of pair n, so the PE never waits on ACT/DVE
    and its HAM clock can ramp to 2.4 GHz.
  - Per q block the accumulated [O^T; l] PSUM is transposed back on the PE
    and normalized by 1/l on DVE, then DMA'd out (rows indexed by j; the
    host gather maps (core, j) -> global block).
"""

import numpy as np

import concourse.bacc as bacc
import concourse.bass as bass
import concourse.mybir as mybir
import concourse.tile as tile
from concourse.bass_utils import run_bass_kernel_spmd
from concourse.masks import make_identity

B, T, C, H = 4, 4096, 1024, 64
NCORES = 8
TB = 512                 # q/t block width
NTB = T // TB            # 8 t-blocks
NQB = 4                  # owned q blocks per core (slot index j)
NKVT = T // 128          # 32 kv tiles of 128
NPAIRS = [4, 8, 12, 16]  # kv pairs per slot j (kv tiles 0..8j+8)
OWN = [[0, 3, 4, 7], [1, 2, 5, 6]]  # global q block per (parity, j)
F32 = mybir.dt.float32
F16 = mybir.dt.float16

_nc = None


def _build():
    nc = bacc.Bacc("TRN2", target_bir_lowering=False, debug=False, num_devices=NCORES)
    xt = nc.dram_tensor("xt", [C, T], F32, kind="ExternalInput").ap()
    wq = nc.dram_tensor("wq", [128, 8 * H], F32, kind="ExternalInput").ap()
    wkv = nc.dram_tensor("wkv", [128, 8 * 2 * H], F32, kind="ExternalInput").ap()
    # per-core resolved: slot parity x 8 tiles x [128, TB] causal masks
    masks = nc.dram_tensor("masks", [128, 16, TB], F16, kind="ExternalInput").ap()
    out = nc.dram_tensor("out", [NQB * TB, H], F32, kind="ExternalOutput").ap()

    with tile.TileContext(nc) as tc:
        pid = nc.partition_id(engines=[mybir.EngineType.PE])
        s = pid % 2
        s1 = (pid + 1) % 2
        # x-piece offset for Q proj of slot j: ((j+s)%2)*TB
        roff_q = [s * TB, s1 * TB]
        with tc.tile_pool(name="persist", bufs=1) as persist, \
             tc.tile_pool(name="x32p", bufs=6) as x32p, \
             tc.tile_pool(name="x16p", bufs=16) as x16p, \
             tc.tile_pool(name="vtp", bufs=2) as vtp, \
             tc.tile_pool(name="otp", bufs=2) as otp, \
             tc.tile_pool(name="obp", bufs=3) as obp, \
             tc.tile_pool(name="rcp", bufs=2) as rcp, \
             tc.tile_pool(name="ptp", bufs=4) as ptp, \
             tc.tile_pool(name="pjp", bufs=2, space="PSUM") as pj_pool, \
             tc.tile_pool(name="psp", bufs=2, space="PSUM") as ps_pool, \
             tc.tile_pool(name="pop", bufs=2, space="PSUM") as po_pool:
            ident = persist.tile([128, 128], F32)
            make_identity(nc, ident)
            wq_sb32 = persist.tile([128, 8 * H], F32)
            wkv_sb32 = persist.tile([128, 8 * 2 * H], F32)
            nc.scalar.dma_start(out=wq_sb32, in_=wq)
            nc.scalar.dma_start(out=wkv_sb32, in_=wkv)
            wq_sb = persist.tile([128, 8 * H], F16)
            wkv_sb = persist.tile([128, 8 * 2 * H], F16)
            nc.vector.tensor_copy(wq_sb, wq_sb32)
            nc.vector.tensor_copy(wkv_sb, wkv_sb32)
            masks_sb = persist.tile([128, 16, TB], F16)
            # chunked so the first x pieces win the DMA queues
            nc.scalar.dma_start(out=masks_sb[:, 0:8, :], in_=masks[:, 0:8, :])

            QT = persist.tile([64, NQB * TB], F16)     # Q^T packed by slot j
            KT = persist.tile([64, T], F16)            # K^T on partitions 0:64
            V = persist.tile([128, NKVT, H + 1], F16)  # [128, 65] per kv tile
            # col 64 of each kv tile = 1.0 (row-sum column)
            nc.scalar.activation(
                V[:, :, H],
                ident[:, 0:NKVT],
                mybir.ActivationFunctionType.Copy,
                scale=0.0,
                bias=1.0,
            )

            # pre-warm the PE clock while the first x DMAs are in flight
            for w in range(24):
                psum_warm = pj_pool.tile([128, TB], F32, name="psum_warm",
                                         tag="pj")
                nc.tensor.matmul(
                    psum_warm[:, 0:128], ident, ident, start=True, stop=True
                )

            x16s = [None] * 8

            # ---- software-pipelined attention emission ------------------
            st = {"po": [None] * NQB, "started": [False] * NQB}
            pending = []

            def emit_s_pair(j, p):
                ps2 = ps_pool.tile([128, 2, TB], F32, name="ps2", tag="ps")
                for h in range(2):
                    k = 2 * p + h
                    nc.tensor.matmul(
                        ps2[:, h, :],
                        KT[:, k * 128:(k + 1) * 128],
                        QT[:, j * TB:(j + 1) * TB],
                        start=True,
                        stop=True,
                    )
                pending.append((j, p, ps2))

            def drain_one():
                j, p, ps2 = pending.pop(0)
                pt2 = ptp.tile([128, 2, TB], F16, name="pt", tag="pt")
                nc.scalar.activation(
                    pt2, ps2, mybir.ActivationFunctionType.Exp, scale=0.125
                )
                if p >= 4 * j:
                    # last-8 kv tiles of this block: causal mask (also zeroes
                    # the over-computed tiles on the smaller-parity core)
                    m2 = (j % 2) * 8 + (p - 4 * j) * 2
                    nc.vector.tensor_mul(
                        pt2, pt2, masks_sb[:, m2:m2 + 2, :]
                    )
                if not st["started"][j]:
                    st["po"][j] = po_pool.tile([H + 1, TB], F32, name="psum_o",
                                               tag="po")
                    st["started"][j] = True
                psum_o = st["po"][j]
                for h in range(2):
                    k = 2 * p + h
                    nc.tensor.matmul(
                        psum_o,
                        V[:, k, :],
                        pt2[:, h, :],
                        start=(k == 0),
                        stop=(k == 2 * NPAIRS[j] - 1),
                    )
                if p == NPAIRS[j] - 1:
                    epilogue(j, psum_o)

            def epilogue(j, psum_o):
                ot = otp.tile([H + 1, TB], F32)
                nc.scalar.copy(ot, psum_o)
                for j2 in range(4):
                    psum_t = pj_pool.tile([128, TB], F32, name="psum_t",
                                          tag="pj")
                    nc.tensor.transpose(
                        psum_t[:, 0:H + 1],
                        ot[:, j2 * 128:(j2 + 1) * 128],
                        ident[0:H + 1, 0:H + 1],
                    )
                    rec = rcp.tile([128, 1], F32)
                    nc.vector.reciprocal(rec, psum_t[:, H:H + 1])
                    ob = obp.tile([128, H], F32)
                    nc.vector.tensor_scalar_mul(ob, psum_t[:, 0:H], rec)
                    nc.sync.dma_start(
                        out=out[j * TB + j2 * 128:j * TB + (j2 + 1) * 128, :],
                        in_=ob,
                    )

            next_p = [0] * NQB
            # Q_j ready after proj t-block: j=3's x is pre-fetched at g=3 so
            # its 16-pair burst spreads over g=3..7 instead of all-after-7
            qready = [1, 3, 5, 3]

            def emit_avail(g):
                for j in range(NQB):
                    if g < qready[j]:
                        continue
                    while (next_p[j] < NPAIRS[j]
                           and (2 * next_p[j] + 1) // 4 <= g):
                        emit_s_pair(j, next_p[j])
                        next_p[j] += 1
                        while len(pending) > 1:
                            drain_one()

            # ---- fused projection + attention stream --------------------
            PIECE = 2 * TB  # 1024: one piece covers t-blocks g, g+1
            for g in range(NTB):
                if g % 2 == 0:
                    p0 = g * TB
                    for c in range(8):
                        x32 = x32p.tile([128, PIECE], F32, name="x32", tag="x32")
                        nc.sync.dma_start(
                            out=x32,
                            in_=xt[c * 128:(c + 1) * 128, p0:p0 + PIECE],
                        )
                        x16 = x16p.tile([128, PIECE], F16, name="x16", tag="x16")
                        nc.vector.tensor_copy(x16, x32)
                        x16s[c] = x16
                if g == 1:
                    nc.scalar.dma_start(
                        out=masks_sb[:, 8:16, :], in_=masks[:, 8:16, :]
                    )
                sl = slice((g % 2) * TB, (g % 2 + 1) * TB)
                psum_vk = pj_pool.tile([128, TB], F32, name="psum_vk", tag="pj")
                for c in range(8):
                    nc.tensor.matmul(
                        psum_vk,
                        wkv_sb[:, c * 128:(c + 1) * 128],
                        x16s[c][:, sl],
                        start=(c == 0),
                        stop=(c == 7),
                    )
                nc.vector.tensor_copy(KT[:, g * TB:(g + 1) * TB], psum_vk[0:64, :])
                vt = vtp.tile([128, TB], F32)
                nc.vector.tensor_copy(vt[64:128, :], psum_vk[64:128, :])
                if g % 2 == 1 and g != 7:
                    j = (g - 1) // 2
                    psum_q = pj_pool.tile([64, TB], F32, name="psum_q", tag="pj")
                    for c in range(8):
                        nc.tensor.matmul(
                            psum_q,
                            wq_sb[:, c * H:(c + 1) * H],
                            x16s[c][:, bass.ds(roff_q[j % 2], TB)],
                            start=(c == 0),
                            stop=(c == 7),
                        )
                    nc.vector.tensor_copy(QT[:, j * TB:(j + 1) * TB], psum_q)
                if g == 3:
                    # pre-fetch the last t-pair's x and project Q_3 now, so
                    # the j=3 attention burst is not serialized after the
                    # final proj block
                    xq16s = []
                    for c in range(8):
                        xq32 = x32p.tile([128, PIECE], F32, name="x32", tag="x32")
                        nc.sync.dma_start(
                            out=xq32,
                            in_=xt[c * 128:(c + 1) * 128, 6 * TB:8 * TB],
                        )
                        xq16 = x16p.tile([128, PIECE], F16, name="x16",
                                         tag="x16")
                        nc.vector.tensor_copy(xq16, xq32)
                        xq16s.append(xq16)
                    psum_q = pj_pool.tile([64, TB], F32, name="psum_q", tag="pj")
                    for c in range(8):
                        nc.tensor.matmul(
                            psum_q,
                            wq_sb[:, c * H:(c + 1) * H],
                            xq16s[c][:, bass.ds(roff_q[1], TB)],
                            start=(c == 0),
                            stop=(c == 7),
                        )
                    nc.vector.tensor_copy(QT[:, 3 * TB:4 * TB], psum_q)
                for j2 in range(4):
                    psum_v = pj_pool.tile([128, H], F32, name="psum_v", tag="pj")
                    nc.tensor.transpose(
                        psum_v,
                        vt[64:128, j2 * 128:(j2 + 1) * 128],
                        ident[64:128, 64:128],
                    )
                    nc.vector.tensor_copy(V[:, 4 * g + j2, 0:H], psum_v)
                emit_avail(g)
            while pending:
                drain_one()

    nc.compile()
    return nc


def get_nc():
    global _nc
    if _nc is None:
        _nc = _build()
    return _nc


def make_inputs(x, Wq, Wk, Wv):
    """Build the 8 per-core input maps."""
    x = np.asarray(x, dtype=np.float32)

    def pack_w(wt):
        # [C, M] (= W.T) -> [128, 8*M]: partition p, free c*M+m = wt[c*128+p, m]
        M = wt.shape[1]
        return np.ascontiguousarray(
            wt.reshape(8, 128, M).transpose(1, 0, 2).reshape(128, 8 * M)
        )

    wq_in = pack_w(np.asarray(Wq, np.float32).T)
    wkv_in = pack_w(
        np.concatenate(
            [np.asarray(Wk, np.float32).T, np.asarray(Wv, np.float32).T], axis=1
        )
    )
    p = np.arange(128, dtype=np.int64)[:, None]
    f = np.arange(TB, dtype=np.int64)[None, :]
    # bank A (q offset 0 in pair) tiles m=0..7, bank B (q offset 512) same
    banks = []
    for qoff in (0, TB):
        banks.append(
            np.stack(
                [((qoff + f - 128 * m - p) >= 0).astype(np.float16)
                 for m in range(8)],
                axis=1,
            )
        )
    # per-parity resolved: slot j uses bank (j+s)%2, stored at (j%2)*8
    masks_by_s = []
    for sp in range(2):
        masks_by_s.append(
            np.ascontiguousarray(
                np.concatenate(
                    [banks[sp], banks[1 - sp]], axis=1
                ).reshape(128, 16 * TB)
            )
        )
    in_maps = []
    for core in range(NCORES):
        b, sp = core // 2, core % 2
        in_maps.append(
            {
                "xt": np.ascontiguousarray(x[b].T),
                "wq": wq_in,
                "wkv": wkv_in,
                "masks": masks_by_s[sp],
            }
        )
    return in_maps


def gather_output(results):
    """results: list of per-core {"out": [2048, 64]} -> full [B, T, H]."""
    O = np.empty((B, T, H), np.float32)
    for core in range(NCORES):
        b, s = core // 2, core % 2
        o = results[core]["out"]
        for j in range(NQB):
            g = OWN[s][j]
            O[b, g * TB:(g + 1) * TB] = o[j * TB:(j + 1) * TB]
    return O


def kernel(x, Wq, Wk, Wv):
    nc = get_nc()
    in_maps = make_inputs(x, Wq, Wk, Wv)
    res = run_bass_kernel_spmd(nc, in_maps, list(range(NCORES)))
    return gather_output(res.results)


# revision 22
# speedup vs baseline: 1.2951x; 1.2951x over previous
"""Causal single-head attention (B=4, T=4096, C=1024, H=64) on 8 TRN2 NeuronCores.

Sharding: 2 cores per batch element. Core s of a pair owns q blocks
OWN[s] = {0,3,4,7} / {1,2,5,6} (512 rows each), which balances the causal
workload (72 useful kv tiles per core) while keeping the SPMD instruction
stream identical across cores: slot index j handles global block OWN[s][j],
and the two parities' blocks for a given j always live in the same 1024-wide
x piece, so a single register offset ((j+s)%2)*512 selects the right Q slice
and causal-mask bank.

One SPMD program for all 8 cores:
  - x arrives pre-transposed per batch as [C, T]; loaded as [128, 1024]
    pieces, cast f32->f16 on the Pool engine (DVE/ACT are loaded).
  - K|V projection runs for the full batch on both cores of a pair; Q is
    projected only for the 4 owned blocks (register-selected x slice).
  - Attention computes S^T = K_tile^T @ Q per 128-wide kv tile: no max pass
    (scores bounded), no P transpose, row-sum folded into P@V via a ones
    column in V. exp runs on ACT over TWO kv tiles at once ([128, 2, 512]
    PSUM spanning 2 banks) with the 1/sqrt(H) scale fused; causal masking is
    a multiplicative f16 mask on the last 8 kv tiles of each q block, bank
    chosen by the (j+s)%2 register.
  - Emission is software-pipelined: the S matmuls for kv pair n+1 are issued
    before the exp/mask/PV drain of pair n, so the PE never waits on ACT/DVE
    and its HAM clock can ramp to 2.4 GHz.
  - Per q block the accumulated [O^T; l] PSUM is transposed back on the PE
    and normalized by 1/l on DVE, then DMA'd out (rows indexed by j; the
    host gather maps (core, j) -> global block).
"""

import numpy as np

import concourse.bacc as bacc
import concourse.bass as bass
import concourse.mybir as mybir
import concourse.tile as tile
from concourse.bass_utils import run_bass_kernel_spmd
from concourse.masks import make_identity

B, T, C, H = 4, 4096, 1024, 64
NCORES = 8
TB = 512                 # q/t block width
NTB = T // TB            # 8 t-blocks
NQB = 4                  # owned q blocks per core (slot index j)
NKVT = T // 128          # 32 kv tiles of 128
NPAIRS = [4, 8, 12, 16]  # kv pairs per slot j (kv tiles 0..8j+8)
OWN = [[0, 3, 4, 7], [1, 2, 5, 6]]  # global q block per (parity, j)
F32 = mybir.dt.float32
F16 = mybir.dt.float16

_nc = None


def _build():
    nc = bacc.Bacc("TRN2", target_bir_lowering=False, debug=False, num_devices=NCORES)
    xt = nc.dram_tensor("xt", [C, T], F16, kind="ExternalInput").ap()
    wq = nc.dram_tensor("wq", [128, 8 * H], F32, kind="ExternalInput").ap()
    wkv = nc.dram_tensor("wkv", [128, 8 * 2 * H], F32, kind="ExternalInput").ap()
    # per-core resolved: slot parity x 8 tiles x [128, TB] causal masks
    masks = nc.dram_tensor("masks", [128, 16, TB], F16, kind="ExternalInput").ap()
    out = nc.dram_tensor("out", [NQB * TB, H], F32, kind="ExternalOutput").ap()

    with tile.TileContext(nc) as tc:
        pid = nc.partition_id(engines=[mybir.EngineType.PE])
        s = pid % 2
        s1 = (pid + 1) % 2
        # x-piece offset for Q proj of slot j: ((j+s)%2)*TB
        roff_q = [s * TB, s1 * TB]
        with tc.tile_pool(name="persist", bufs=1) as persist, \
             tc.tile_pool(name="x16p", bufs=16) as x16p, \
             tc.tile_pool(name="vtp", bufs=2) as vtp, \
             tc.tile_pool(name="otp", bufs=2) as otp, \
             tc.tile_pool(name="obp", bufs=3) as obp, \
             tc.tile_pool(name="rcp", bufs=2) as rcp, \
             tc.tile_pool(name="ptp", bufs=4) as ptp, \
             tc.tile_pool(name="pjp", bufs=2, space="PSUM") as pj_pool, \
             tc.tile_pool(name="psp", bufs=2, space="PSUM") as ps_pool, \
             tc.tile_pool(name="pop", bufs=2, space="PSUM") as po_pool:
            ident = persist.tile([128, 128], F32)
            make_identity(nc, ident)
            wq_sb32 = persist.tile([128, 8 * H], F32)
            wkv_sb32 = persist.tile([128, 8 * 2 * H], F32)
            nc.scalar.dma_start(out=wq_sb32, in_=wq)
            nc.scalar.dma_start(out=wkv_sb32, in_=wkv)
            wq_sb = persist.tile([128, 8 * H], F16)
            wkv_sb = persist.tile([128, 8 * 2 * H], F16)
            nc.vector.tensor_copy(wq_sb, wq_sb32)
            nc.vector.tensor_copy(wkv_sb, wkv_sb32)
            masks_sb = persist.tile([128, 16, TB], F16)
            # chunked so the first x pieces win the DMA queues
            nc.scalar.dma_start(out=masks_sb[:, 0:8, :], in_=masks[:, 0:8, :])

            QT = persist.tile([64, NQB * TB], F16)     # Q^T packed by slot j
            KT = persist.tile([64, T], F16)            # K^T on partitions 0:64
            V = persist.tile([128, NKVT, H + 1], F16)  # [128, 65] per kv tile
            # col 64 of each kv tile = 1.0 (row-sum column)
            nc.scalar.activation(
                V[:, :, H],
                ident[:, 0:NKVT],
                mybir.ActivationFunctionType.Copy,
                scale=0.0,
                bias=1.0,
            )

            # pre-warm the PE clock while the first x DMAs are in flight
            for w in range(24):
                psum_warm = pj_pool.tile([128, TB], F32, name="psum_warm",
                                         tag="pj")
                nc.tensor.matmul(
                    psum_warm[:, 0:128], ident, ident, start=True, stop=True
                )

            x16s = [None] * 8

            # ---- software-pipelined attention emission ------------------
            st = {"po": [None] * NQB, "started": [False] * NQB}
            pending = []

            def emit_s_pair(j, p):
                ps2 = ps_pool.tile([128, 2, TB], F32, name="ps2", tag="ps")
                for h in range(2):
                    k = 2 * p + h
                    nc.tensor.matmul(
                        ps2[:, h, :],
                        KT[:, k * 128:(k + 1) * 128],
                        QT[:, j * TB:(j + 1) * TB],
                        start=True,
                        stop=True,
                    )
                pending.append((j, p, ps2))

            def drain_one():
                j, p, ps2 = pending.pop(0)
                pt2 = ptp.tile([128, 2, TB], F16, name="pt", tag="pt")
                nc.scalar.activation(
                    pt2, ps2, mybir.ActivationFunctionType.Exp, scale=0.125
                )
                if p >= 4 * j:
                    # last-8 kv tiles of this block: causal mask (also zeroes
                    # the over-computed tiles on the smaller-parity core)
                    m2 = (j % 2) * 8 + (p - 4 * j) * 2
                    nc.vector.tensor_mul(
                        pt2, pt2, masks_sb[:, m2:m2 + 2, :]
                    )
                if not st["started"][j]:
                    st["po"][j] = po_pool.tile([H + 1, TB], F32, name="psum_o",
                                               tag="po")
                    st["started"][j] = True
                psum_o = st["po"][j]
                for h in range(2):
                    k = 2 * p + h
                    nc.tensor.matmul(
                        psum_o,
                        V[:, k, :],
                        pt2[:, h, :],
                        start=(k == 0),
                        stop=(k == 2 * NPAIRS[j] - 1),
                    )
                if p == NPAIRS[j] - 1:
                    epilogue(j, psum_o)

            def epilogue(j, psum_o):
                ot = otp.tile([H + 1, TB], F32)
                nc.scalar.copy(ot, psum_o)
                for j2 in range(4):
                    psum_t = pj_pool.tile([128, TB], F32, name="psum_t",
                                          tag="pj")
                    nc.tensor.transpose(
                        psum_t[:, 0:H + 1],
                        ot[:, j2 * 128:(j2 + 1) * 128],
                        ident[0:H + 1, 0:H + 1],
                    )
                    rec = rcp.tile([128, 1], F32)
                    nc.vector.reciprocal(rec, psum_t[:, H:H + 1])
                    ob = obp.tile([128, H], F32)
                    nc.vector.tensor_scalar_mul(ob, psum_t[:, 0:H], rec)
                    nc.sync.dma_start(
                        out=out[j * TB + j2 * 128:j * TB + (j2 + 1) * 128, :],
                        in_=ob,
                    )

            next_p = [0] * NQB
            # Q_j ready after proj t-block: j=3's x is pre-fetched at g=3 so
            # its 16-pair burst spreads over g=3..7 instead of all-after-7
            qready = [1, 3, 5, 7]

            def emit_avail(g):
                for j in range(NQB):
                    if g < qready[j]:
                        continue
                    while (next_p[j] < NPAIRS[j]
                           and (2 * next_p[j] + 1) // 4 <= g):
                        emit_s_pair(j, next_p[j])
                        next_p[j] += 1
                        while len(pending) > 1:
                            drain_one()

            # ---- fused projection + attention stream --------------------
            PIECE = 2 * TB  # 1024: one piece covers t-blocks g, g+1
            for g in range(NTB):
                if g % 2 == 0:
                    p0 = g * TB
                    for c in range(8):
                        x16 = x16p.tile([128, PIECE], F16, name="x16", tag="x16")
                        nc.sync.dma_start(
                            out=x16,
                            in_=xt[c * 128:(c + 1) * 128, p0:p0 + PIECE],
                        )
                        x16s[c] = x16
                if g == 1:
                    nc.scalar.dma_start(
                        out=masks_sb[:, 8:16, :], in_=masks[:, 8:16, :]
                    )
                sl = slice((g % 2) * TB, (g % 2 + 1) * TB)
                psum_vk = pj_pool.tile([128, TB], F32, name="psum_vk", tag="pj")
                for c in range(8):
                    nc.tensor.matmul(
                        psum_vk,
                        wkv_sb[:, c * 128:(c + 1) * 128],
                        x16s[c][:, sl],
                        start=(c == 0),
                        stop=(c == 7),
                    )
                nc.vector.tensor_copy(KT[:, g * TB:(g + 1) * TB], psum_vk[0:64, :])
                vt = vtp.tile([128, TB], F32)
                nc.vector.tensor_copy(vt[64:128, :], psum_vk[64:128, :])
                if g % 2 == 1:
                    j = (g - 1) // 2
                    psum_q = pj_pool.tile([64, TB], F32, name="psum_q", tag="pj")
                    for c in range(8):
                        nc.tensor.matmul(
                            psum_q,
                            wq_sb[:, c * H:(c + 1) * H],
                            x16s[c][:, bass.ds(roff_q[j % 2], TB)],
                            start=(c == 0),
                            stop=(c == 7),
                        )
                    nc.vector.tensor_copy(QT[:, j * TB:(j + 1) * TB], psum_q)
                for j2 in range(4):
                    psum_v = pj_pool.tile([128, H], F32, name="psum_v", tag="pj")
                    nc.tensor.transpose(
                        psum_v,
                        vt[64:128, j2 * 128:(j2 + 1) * 128],
                        ident[64:128, 64:128],
                    )
                    nc.vector.tensor_copy(V[:, 4 * g + j2, 0:H], psum_v)
                emit_avail(g)
            while pending:
                drain_one()

    nc.compile()
    return nc


def get_nc():
    global _nc
    if _nc is None:
        _nc = _build()
    return _nc


def make_inputs(x, Wq, Wk, Wv):
    """Build the 8 per-core input maps."""
    x = np.asarray(x, dtype=np.float32)

    def pack_w(wt):
        # [C, M] (= W.T) -> [128, 8*M]: partition p, free c*M+m = wt[c*128+p, m]
        M = wt.shape[1]
        return np.ascontiguousarray(
            wt.reshape(8, 128, M).transpose(1, 0, 2).reshape(128, 8 * M)
        )

    wq_in = pack_w(np.asarray(Wq, np.float32).T)
    wkv_in = pack_w(
        np.concatenate(
            [np.asarray(Wk, np.float32).T, np.asarray(Wv, np.float32).T], axis=1
        )
    )
    p = np.arange(128, dtype=np.int64)[:, None]
    f = np.arange(TB, dtype=np.int64)[None, :]
    # bank A (q offset 0 in pair) tiles m=0..7, bank B (q offset 512) same
    banks = []
    for qoff in (0, TB):
        banks.append(
            np.stack(
                [((qoff + f - 128 * m - p) >= 0).astype(np.float16)
                 for m in range(8)],
                axis=1,
            )
        )
    # per-parity resolved: slot j uses bank (j+s)%2, stored at (j%2)*8
    masks_by_s = []
    for sp in range(2):
        masks_by_s.append(
            np.ascontiguousarray(
                np.concatenate(
                    [banks[sp], banks[1 - sp]], axis=1
                ).reshape(128, 16 * TB)
            )
        )
    in_maps = []
    for core in range(NCORES):
        b, sp = core // 2, core % 2
        in_maps.append(
            {
                "xt": np.ascontiguousarray(x[b].T.astype(np.float16)),
                "wq": wq_in,
                "wkv": wkv_in,
                "masks": masks_by_s[sp],
            }
        )
    return in_maps


def gather_output(results):
    """results: list of per-core {"out": [2048, 64]} -> full [B, T, H]."""
    O = np.empty((B, T, H), np.float32)
    for core in range(NCORES):
        b, s = core // 2, core % 2
        o = results[core]["out"]
        for j in range(NQB):
            g = OWN[s][j]
            O[b, g * TB:(g + 1) * TB] = o[j * TB:(j + 1) * TB]
    return O


def kernel(x, Wq, Wk, Wv):
    nc = get_nc()
    in_maps = make_inputs(x, Wq, Wk, Wv)
    res = run_bass_kernel_spmd(nc, in_maps, list(range(NCORES)))
    return gather_output(res.results)
